# revision 21
# baseline (speedup 1.0000x reference)
"""GAT layer (nn_GATLayerAdj) Trainium2 Bass kernel, 8-core SPMD — v3.

Reference computation (N=1024, di=do=64):
    a[i,j]  = x[j]@w_src + x[i]@w_tgt + bw        (attention logits)
    att     = softmax_j(where(adj>0, a, -1e16))
    y[i,j,:]= relu(x[j]@WfS.T + x[i]@WfT.T + bf)
    o[i,:]  = sum_j att[i,j] * y[i,j,:]

Sharding: target-node dim i split across 8 cores (128 rows each).

Algebraic restructurings (vs the ~74us v1):
  1. Row-softmax is invariant to the per-row shift a_tgt[i]+bw, so the
     attention weights reduce to eT[j,i] = exp(a_src[j]) * adjT[j,i] —
     one per-partition ACT scale-copy per j-chunk, directly in
     TRANSPOSED layout (j on partitions): no logits outer product, no
     big exp, no PE transposes.
  2. relu(ys_j + u_i) = max(ys_j, -u_i) + u_i. Most of the N^2*do/8
     elementwise volume is ONE DVE tensor_tensor max per half-chunk
     (bf16 2x mode); v1 needed an add + a relu. The +u_i correction is
     applied at evacuation: o = t_acc*(1/s) - nurep2*(sM/s), where
     nurep2 is a diagonal-replicated -u tile (4 partition-broadcast
     DMAs) and sM is the partial attention row-sum over max-form
     chunks (mask-weighted reduce of per-chunk row sums).
  3. A few half-chunks run in relu-form on otherwise-idle engines:
     Pool computes z = ys - (-u) (TensorTensor subtract is the one
     elementwise op its Q7 ucode supports; max is not), ACT evacuates
     relu(z). Those (chunk, half)s are excluded from sM via a 0/1 mask
     built with memsets, so the evacuation correction stays exact.

Engine budget per core (~measured): DVE 13 maxes ~30us (the wall),
Pool 3 subtracts ~25us, ACT relus+small ops ~19us, PE reduce
(4x4 col-tiled accumulating matmuls, tile_position groups run
concurrently) ~24us of slices at ~3x overlap.

Numerics: bf16 inputs, fp32 PSUM accumulation; max keeps one operand
exact, u enters through the same bf16 rounding as v1.
"""

from contextlib import ExitStack

import numpy as np
import ml_dtypes

import concourse.bass as bass
import concourse.tile as tile
from concourse import bacc, mybir
from concourse.bass_utils import run_bass_kernel_spmd

# Lighter TileContext exit: stock emits drain + full butterfly barrier +
# sem clears + second butterfly (~11us). Engines already sync at program
# end; keep the drain (output DMA completion), a sem-only rendezvous
# before the clears, and drop the trailing barrier.
import concourse.tile as _tile_mod

if not getattr(_tile_mod, "_exit_trimmed", False):
    def _drain_and_barrier_trim(self, tick_clock, wait_clock):
        from concourse.tile import ScopedClock
        nc = self.nc
        drain_inst = nc.sync.drain()
        wait_clock.add_sem_waits(
            drain_inst.ins, ScopedClock({None: tick_clock.global_clock})
        )
        # parallel rendezvous: every engine incs one sem; gpsimd waits,
        # clears the tile sems, and the program ends (engines sync at
        # program completion anyway - no trailing butterfly needed)
        exit_sem = nc.alloc_semaphore("exit_rdv")
        for eng in (nc.sync, nc.tensor, nc.vector, nc.scalar):
            eng.nop(nofuse=True).then_inc(exit_sem, 1)
        nc.gpsimd.wait_ge(exit_sem, 4)
        assert self.sems is not None
        popped = nc._tile_sem_poison_stack.pop()
        assert popped is self._sem_poison
        nc.clear_and_free_semaphores(list(self.sems.allocated().values()))
        nc.gpsimd.sem_clear(range(exit_sem.num, exit_sem.num + 1))

    _tile_mod.TileContext._drain_and_barrier = _drain_and_barrier_trim
    _tile_mod._exit_trimmed = True

N = 1024
DI = 64
DO = 64
N_CORES = 8
ROWS = N // N_CORES          # 128 target rows per core
NCHUNK = N // 128            # 8 j-chunks
F_FULL = ROWS * DO           # 8192 free size of (i, d)
HALF = F_FULL // 2           # 4096: half-chunk unit
QUART = F_FULL // 4          # 2048

f32 = mybir.dt.float32
bf16 = mybir.dt.bfloat16
AF = mybir.ActivationFunctionType
ALU = mybir.AluOpType
AX = mybir.AxisListType

# (chunk, half) pairs computed in relu-form: Pool subtract + ACT relu.
# Measured: Pool TensorTensor is ~10us per half AND its SBUF traffic
# slows concurrent DVE maxes ~4x — strictly a loss. Keep empty.
POOL_R_HALVES = frozenset()

_CACHE = {}


def _build_program():
    nc = bacc.Bacc("TRN2", target_bir_lowering=False, debug=False,
                   num_devices=N_CORES)

    # ---- DRAM I/O (concatenated to cut ~600ns-per-trigger DMA issue) ----
    # xw = [xbTa | nwfta]  [65, 192], xm = [xT | wfsT | ws]  [64, 1089]
    xw_d = nc.dram_tensor("xw", [DI + 1, ROWS + DO], bf16,
                          kind="ExternalInput").ap()
    xm_d = nc.dram_tensor("xm", [DI, N + DO + 1], bf16,
                          kind="ExternalInput").ap()
    adjT_d = nc.dram_tensor("adjT", [ROWS, N], bf16, kind="ExternalInput").ap()
    o_d = nc.dram_tensor("o", [128, 2048], f32, kind="ExternalOutput").ap()

    with tile.TileContext(nc) as tc, ExitStack() as ctx:
        cons = ctx.enter_context(tc.tile_pool(name="cons", bufs=1))
        rp = ctx.enter_context(tc.tile_pool(name="rp", bufs=5))
        zp = ctx.enter_context(tc.tile_pool(name="zp", bufs=2))
        psp = ctx.enter_context(tc.tile_pool(name="psp", bufs=2, space="PSUM"))
        accs = ctx.enter_context(tc.tile_pool(name="accs", bufs=1, space="PSUM"))
        accr = ctx.enter_context(tc.tile_pool(name="accr", bufs=1, space="PSUM"))
        accp = ctx.enter_context(tc.tile_pool(name="accp", bufs=1, space="PSUM"))

        # ---- input DMAs: u-chain tensor first ----
        xw_t = cons.tile([DI + 1, ROWS + DO], bf16)
        nc.sync.dma_start(xw_t[:], xw_d[:, :])
        xm_t = cons.tile([DI, N + DO + 1], bf16)
        nc.sync.dma_start(xm_t[:], xm_d[:, :])
        xbTa_t = xw_t[:, :ROWS]
        nwfta_t = xw_t[:, ROWS:ROWS + DO]
        xT_t = xm_t[:, :N]
        wfsT_t = xm_t[:, N:N + DO]
        ws_t = xm_t[:, N + DO:N + DO + 1]
        adjT_t = cons.tile([ROWS, N], bf16)
        nc.gpsimd.dma_start(adjT_t[:], adjT_d[:, :])

        # ---- nu = -(xb@WfT.T + bf)  [128, 64] (K=65 ones-row trick) ----
        # The whole chain gates the DVE maxes (the critical engine), so
        # it runs at scheduler priority 0. Quarters go on the sync +
        # gpsimd queues only: triggers issued from the ACT engine get
        # scheduled behind its compute ops.
        nurep = cons.tile([128, F_FULL], bf16)
        nu_dram = nc.dram_tensor("nu_stage", [F_FULL], bf16).ap()
        nu_sb = cons.tile([ROWS, DO], bf16)
        with tc.high_priority():
            nu_ps = psp.tile([ROWS, DO], f32, tag="pre")
            nc.tensor.matmul(nu_ps[:], xbTa_t, nwfta_t, start=True, stop=True)
            nc.scalar.copy(nu_sb[:], nu_ps[:])
            nc.sync.dma_start(out=nu_dram.rearrange("(i d) -> i d", i=ROWS),
                              in_=nu_sb[:, :])
            # broadcast pieces: a fat half on the sync ring (8KB
            # descriptors run the ring at ~345GB/s) + a quarter each on
            # sync and gpsimd. No DMA on the ACT queue (its triggers get
            # scheduled behind compute).
            for off, ln, eng in ((0, HALF, nc.sync),
                                 (3 * QUART, QUART, nc.gpsimd),
                                 (2 * QUART, QUART, nc.sync)):
                sl = slice(off, off + ln)
                src = nu_dram[sl]
                bsrc = bass.AP(tensor=src.tensor, offset=src.offset,
                               ap=[[0, 128]] + [list(d) for d in src.ap])
                eng.dma_start(out=nurep[:, sl], in_=bsrc)
        # flat -u row (rank-1 +u fix-up rhs)
        nu_flat = cons.tile([1, F_FULL], bf16)
        nc.sync.dma_start(out=nu_flat[:, :],
                          in_=nu_dram.rearrange("(o f) -> o f", o=1))

        # ---- a_src row + exp -> es, re-laid out per-partition ----
        # (high priority: gates the eT chunks -> ssum -> the reduce's
        # rank-1 opener; the gpsimd queue must run these BEFORE the
        # nurep quarter it also carries)
        es_row = cons.tile([1, N], bf16)
        es_dram = nc.dram_tensor("es_stage", [N], bf16).ap()
        es_col = cons.tile([128, NCHUNK], f32)
        with tc.high_priority():
            for h in range(2):
                hs = slice(512 * h, 512 * (h + 1))
                asp = psp.tile([1, 512], f32, tag="pre", name=f"asp{h}")
                nc.tensor.matmul(asp[:], ws_t, xT_t[:, hs],
                                 start=True, stop=True)
                nc.scalar.activation(es_row[:, hs], asp[:], AF.Exp)
            nc.gpsimd.dma_start(out=es_dram.rearrange("(o f) -> o f", o=1),
                                in_=es_row[:, :])
            # f32: ACT scale APs must be FP32; the gpsimd SWDGE DMA casts
            nc.gpsimd.dma_start(out=es_col[:, :],
                                in_=es_dram.rearrange("(c p) -> p c", p=128))

        # ---- ys chunks: ys_jp[j_local, 64*c + d] = ys[128*c + j_local, d] ----
        ys_jp = cons.tile([128, NCHUNK * DO], bf16)
        for c in range(NCHUNK):
            ysp = psp.tile([128, DO], f32, tag="pre", name=f"ysp{c}")
            nc.tensor.matmul(ysp[:], xT_t[:, 128 * c:128 * (c + 1)], wfsT_t,
                             start=True, stop=True)
            nc.scalar.copy(ys_jp[:, DO * c:DO * (c + 1)], ysp[:])

        # ---- eT chunks (ACT scale-copy) + per-chunk row sums ----
        onescol = cons.tile([128, 1], bf16)
        nc.vector.memset(onescol[:], 1.0)
        et_all = cons.tile([128, N], bf16)
        # two accumulations over the eT chunks: per-target row sums s as
        # a column (for 1/s) AND as a row (negated -> rank-1 fix-up
        # lhsT; no DRAM transpose round trip needed)
        ssum_ps = accs.tile([ROWS, 1], f32, tag="acc")
        s_row_ps = accr.tile([1, ROWS], f32, tag="acc")
        for c in range(NCHUNK):
            cs = slice(128 * c, 128 * (c + 1))
            nc.scalar.activation(et_all[:, cs], adjT_t[:, cs], AF.Copy,
                                 bias=0.0, scale=es_col[:, c:c + 1])
            nc.tensor.matmul(ssum_ps[:], et_all[:, cs], onescol[:],
                             start=(c == 0), stop=(c == NCHUNK - 1),
                             skip_group_check=True)
            nc.tensor.matmul(s_row_ps[:], onescol[:], et_all[:, cs],
                             start=(c == 0), stop=(c == NCHUNK - 1),
                             skip_group_check=True)
        r_t = cons.tile([ROWS, 1], f32)
        nsT = cons.tile([1, ROWS], bf16)
        nc.scalar.activation(nsT[:], s_row_ps[:], AF.Copy, bias=0.0,
                             scale=-1.0)

        # ---- max build + reduce, software-pipelined ----
        t_acc = accp.tile([128, 2048], f32, tag="acc")
        r_tiles = {}

        def emit_build(c):
            r_c = rp.tile([128, F_FULL], bf16, name="r_c")
            r_tiles[c] = r_c
            ys_c = ys_jp[:, DO * c:DO * (c + 1)]
            # c0 at quarter granularity (starts on the first nurep
            # quarter), c7 as halves h1-first (so b2/b3 reduces overlap
            # the h0 max), the rest as single full-chunk ops (less
            # per-instruction overhead).
            if c == 0:
                # piece order matches expected nurep landing order
                parts = [(0, HALF), (3 * QUART, QUART), (2 * QUART, QUART)]
            elif c == NCHUNK - 1:
                parts = [(HALF, HALF), (0, HALF)]
            else:
                parts = [(0, F_FULL)]
            for off, ln in parts:
                sl = slice(off, off + ln)
                rv = r_c[:, sl].rearrange("p (i d) -> p i d", i=ln // DO)
                nuv = nurep[:, sl].rearrange("p (i d) -> p i d", i=ln // DO)
                ys_b = ys_c.rearrange("p d -> p () d").broadcast_to(
                    (128, ln // DO, DO))
                nc.vector.tensor_tensor(rv, ys_b, nuv, ALU.max)

        def emit_reduce(c):
            r_c = r_tiles.pop(c)
            border = (2, 3, 0, 1) if c == NCHUNK - 1 else (0, 1, 2, 3)
            for n2 in range(4):
                for b in border:
                    nc.tensor.matmul(
                        t_acc[32 * b:32 * (b + 1), 512 * n2:512 * (n2 + 1)],
                        et_all[:, 128 * c + 32 * b:128 * c + 32 * (b + 1)],
                        r_c[:, 2048 * b + 512 * n2:2048 * b + 512 * (n2 + 1)],
                        start=False,
                        stop=(c == NCHUNK - 1),
                        skip_group_check=True,
                        tile_position=(0, 32 * b),
                    )

        # rank-1 +u fix-up opens every accumulation region:
        # t_acc[i, (i',d)] starts at (-s_i)*(-u[i',d]) = s_i*u[i',d]
        for n2 in range(4):
            for b in range(4):
                nc.tensor.matmul(
                    t_acc[32 * b:32 * (b + 1), 512 * n2:512 * (n2 + 1)],
                    nsT[:, 32 * b:32 * (b + 1)],
                    nu_flat[:, 2048 * b + 512 * n2:2048 * b + 512 * (n2 + 1)],
                    start=True, stop=False,
                    skip_group_check=True,
                    tile_position=(0, 32 * b),
                )

        SKEW = 2
        for cc in range(NCHUNK + SKEW):
            if cc < NCHUNK:
                emit_build(cc)
            if cc == 3:
                # mid-DVE-queue: ssum is long done by now
                nc.vector.reciprocal(r_t[:], ssum_ps[:])
            if cc >= SKEW:
                emit_reduce(cc - SKEW)

        # ---- tail: o = t_acc*(1/s), ACT/DVE in parallel; DMA out ----
        # separate destination tiles per engine — a shared tile would
        # serialize the writers at tile granularity
        t_sa = cons.tile([128, 1024], f32)
        t_sv = cons.tile([128, 1024], f32)
        for n2 in range(4):
            sl = slice(512 * n2, 512 * (n2 + 1))
            osl = slice(512 * (n2 // 2), 512 * (n2 // 2 + 1))
            if n2 % 2 == 0:
                nc.scalar.activation(t_sa[:, osl], t_acc[:, sl], AF.Copy,
                                     bias=0.0, scale=r_t[:])
                nc.sync.dma_start(o_d[:, sl], t_sa[:, osl])
            else:
                nc.vector.tensor_scalar(t_sv[:, osl], t_acc[:, sl], r_t[:],
                                        None, ALU.mult)
                nc.scalar.dma_start(o_d[:, sl], t_sv[:, osl])

    nc.compile()
    return nc


def _prep_inputs(x, adj, Wf, bf_, Ww, bw):
    b = ml_dtypes.bfloat16
    xT = np.ascontiguousarray(x.T)                        # [64, N]
    wfsT = np.ascontiguousarray(Wf[:, :DI].T)             # [64, 64]
    ws = Ww[0, :DI].reshape(DI, 1)                        # [64, 1]
    xm = np.hstack([xT, wfsT, ws]).astype(b)              # [64, 1089]
    nwfta = -np.vstack([Wf[:, DI:].T, bf_[None, :]])      # [65, 64]

    in_maps = []
    for c in range(N_CORES):
        blk = slice(ROWS * c, ROWS * (c + 1))
        xbTa = np.vstack([x[blk].T, np.ones((1, ROWS), np.float32)])
        xw = np.hstack([xbTa, nwfta]).astype(b)           # [65, 192]
        # adjT chunk-major: adjT[j_loc, 128c + i] = adj[blk0+i, 128c+j_loc]
        adjT = (adj[blk].T.reshape(NCHUNK, 128, ROWS)
                .transpose(1, 0, 2).reshape(128, N))
        in_maps.append(dict(xm=xm, xw=xw,
                            adjT=np.ascontiguousarray(adjT).astype(b)))
    return in_maps


def get_program():
    if "nc" not in _CACHE:
        _CACHE["nc"] = _build_program()
    return _CACHE["nc"]


def kernel(x, adj, Wf, bf, Ww, bw):
    x = np.asarray(x, dtype=np.float32)
    adj = np.asarray(adj, dtype=np.int32)
    Wf = np.asarray(Wf, dtype=np.float32)
    bf_ = np.asarray(bf, dtype=np.float32)
    Ww = np.asarray(Ww, dtype=np.float32)
    bw = np.asarray(bw, dtype=np.float32)
    assert x.shape == (N, DI) and adj.shape == (N, N)

    nc = get_program()
    in_maps = _prep_inputs(x, adj, Wf, bf_, Ww, bw)
    res = run_bass_kernel_spmd(nc, in_maps, core_ids=list(range(N_CORES)))
    p_idx = np.arange(128)
    col0 = (p_idx % 32) * DO
    out = np.empty((N, DO), np.float32)
    for c in range(N_CORES):
        t = res.results[c]["o"]                      # [128, 2048]
        out[ROWS * c:ROWS * (c + 1)] = t[p_idx[:, None],
                                         col0[:, None] + np.arange(DO)[None, :]]
    return out
